# revision 18
# baseline (speedup 1.0000x reference)
"""Causal GQA self-attention on 8 Trainium2 NeuronCores.

Sharding: data-parallel over batch (4) x tensor-parallel over heads (2 halves
of 14 heads each, KV heads replicated for the shared GQA group). Each core
computes a partial output (its heads' contribution through the row-parallel
out-projection); the host sums the two partials per batch element.

Local head layout (per core): 14 heads = 4 kv-groups x up-to-4 heads. Head
slot (g, c) lives at QT partition rows 32g..32g+32, chunk c; its kv group g's
K lives at KT rows 32g (no replication needed: Q and K share the partition
range, tile_position=(32g, 0)). Host permutes Wq columns / Wo rows into
(c-major, g-minor) order so chunk c of QT is a contiguous 128-column block.

Engine plan (cost model: engine time ~ output free-size, partitions free):
  PE:   x transpose (f32r), QKV proj, scores S^T[kpos, q] (bf16), causal mask
        via a -1e5 upper-tri matmul accumulated into the diagonal PSUM block
        pre-exp, AV as out[q, 33] (col 32 = softmax denominator via a ones
        column in V), out-projection.
  ACT:  exp only (PSUM -> SBUF bf16, scale folded).
  DVE:  xT/KT/vts PSUM evictions, reciprocal of denominators.
  Pool: QT evictions, AO normalize (av * 1/z), small memsets.
  DMA:  XBAR transposes for V and the attention output (SBUF->SBUF bf16).
"""

import sys

sys.path.insert(0, "/opt/trn_rl_repo")

from contextlib import ExitStack

import numpy as np

import concourse.bass as bass
import concourse.mybir as mybir
import concourse.tile as tile
from concourse import bacc
from concourse.bass import ts
from concourse.bass_utils import run_bass_kernel_spmd

F32 = mybir.dt.float32
F32R = mybir.dt.float32r
BF16 = mybir.dt.bfloat16
EXP = mybir.ActivationFunctionType.Exp
MUL = mybir.AluOpType.mult
P = 128
T, C = 2048, 896
D = 32
HL = 14  # local heads per core
GL = 4  # local kv groups per core
DH = HL * D  # 448
DKV = GL * D  # 128
SCALE = 1.0 / float(np.sqrt(D))
NEG = -1.0e5  # causal mask additive value (pre-scale)

HEADS_HALF = [
    list(range(0, 12)) + [24, 25],
    list(range(12, 24)) + [26, 27],
]
KV_HALF = [[0, 1, 2, 6], [3, 4, 5, 6]]
# head pairs (g, c0): heads (g, c0) and (g, c0+1)
PAIRS = [(0, 0), (0, 2), (1, 0), (1, 2), (2, 0), (2, 2), (3, 0)]
# rows used per chunk c (chunks 2,3 only have groups 0..2)
CH_ROWS = [128, 128, 96, 96]
# chunk column offsets within the 448 permuted head dims
CH_OFF = [0, 128, 256, 352]


def _trace(tc, d):
    nc = tc.nc
    with ExitStack() as ctx:
        const = ctx.enter_context(tc.tile_pool(name="const", bufs=1))
        identr = const.tile([P, P], F32R)
        nc.sync.dma_start(identr[:], d["identr"][:])
        identb = const.tile([P, P], BF16)
        nc.sync.dma_start(identb[:], d["identb"][:])
        maskT = const.tile([P, P], BF16)
        nc.sync.dma_start(maskT[:], d["masktb"][:])

        persist = ctx.enter_context(tc.tile_pool(name="persist", bufs=1))
        xT = persist.tile([P, 7, T], F32R, tag="xT")
        QT = persist.tile([P, 4, T], BF16, tag="QT")
        KT = persist.tile([P, T], BF16, tag="KT")
        V = persist.tile([P, 16, GL, 33], BF16, tag="V")
        AOT = persist.tile([P, 4, T], BF16, tag="AOT")

        w = ctx.enter_context(tc.tile_pool(name="w", bufs=1))
        WqH = w.tile([P, 7, DH], F32R, tag="WqH")
        nc.sync.dma_start(WqH[:], d["wq"].rearrange("(co ci) n -> ci co n", ci=P))
        WkH = w.tile([P, 7, DKV], F32R, tag="WkH")
        nc.sync.dma_start(WkH[:], d["wk"].rearrange("(co ci) n -> ci co n", ci=P))
        WvH = w.tile([P, 7, DKV], F32R, tag="WvH")
        nc.sync.dma_start(WvH[:], d["wv"].rearrange("(co ci) n -> ci co n", ci=P))
        WoH = w.tile([P, 4, C], BF16, tag="WoH")
        for c in range(4):
            nc.sync.dma_start(
                WoH[: CH_ROWS[c], c, :],
                d["wo"][CH_OFF[c] : CH_OFF[c] + CH_ROWS[c], :],
            )
        # ones column of V (softmax denominator accumulator)
        nc.gpsimd.memset(V[:, :, :, 32:33], 1.0)

        xv = d["x"].rearrange("(to ti) c -> ti to c", ti=P)
        ov = d["out"].rearrange("(to ti) c -> ti to c", ti=P)

        xraw = ctx.enter_context(tc.tile_pool(name="xraw", bufs=8))
        pp = ctx.enter_context(tc.tile_pool(name="pp", bufs=2, space="PSUM"))
        pss = ctx.enter_context(tc.tile_pool(name="pss", bufs=2, space="PSUM"))
        pav = ctx.enter_context(tc.tile_pool(name="pav", bufs=2, space="PSUM"))
        vtt = ctx.enter_context(tc.tile_pool(name="vtt", bufs=2))
        ptp = ctx.enter_context(tc.tile_pool(name="ptp", bufs=3))
        rzp = ctx.enter_context(tc.tile_pool(name="rzp", bufs=2))
        aop = ctx.enter_context(tc.tile_pool(name="aop", bufs=2))
        obp = ctx.enter_context(tc.tile_pool(name="obp", bufs=2))

        def out_proj(qc):
            qs = qc * 512
            for tcl in range(4):
                tg = qc * 4 + tcl
                for half in range(2):
                    po = pp.tile([P, 448], F32, tag="pp")
                    for c in range(4):
                        K = CH_ROWS[c]
                        nc.tensor.matmul(
                            po[:],
                            lhsT=AOT[:K, c, qs + tcl * P : qs + (tcl + 1) * P],
                            rhs=WoH[:K, c, half * 448 : (half + 1) * 448],
                            start=(c == 0),
                            stop=(c == 3),
                        )
                    ob = obp.tile([P, 448], F32, tag="ob")
                    nc.vector.tensor_copy(ob[:], po[:])
                    nc.sync.dma_start(ov[:, tg, half * 448 : (half + 1) * 448], ob[:])

        for step in range(4):
            nk = step
            # ---- x -> xT (PE transpose, f32r) ----
            xt4 = []
            for k in range(4):
                xtile = xraw.tile([P, C], F32R, tag="xtile")
                nc.sync.dma_start(xtile[:], xv[:, 4 * step + k, :])
                xt4.append(xtile)
            for cc in range(7):
                ps = pp.tile([P, 512], F32R, tag="pp")
                for k in range(4):
                    nc.tensor.transpose(ps[:, ts(k, P)], xt4[k][:, ts(cc, P)], identr[:])
                nc.vector.tensor_copy(xT[:, cc, ts(step, 512)], ps[:])

            # ---- projections for this t-chunk (nk) ----
            for c in range(4):
                M = CH_ROWS[c]
                ps = pp.tile([P, 512], F32, tag="pp")
                for cc in range(7):
                    nc.tensor.matmul(
                        ps[:M, :],
                        lhsT=WqH[:, cc, CH_OFF[c] : CH_OFF[c] + M],
                        rhs=xT[:, cc, ts(nk, 512)],
                        start=(cc == 0),
                        stop=(cc == 6),
                    )
                nc.vector.tensor_copy(QT[:M, c, ts(nk, 512)], ps[:M, :])
            ps = pp.tile([P, 512], F32, tag="pp")
            for cc in range(7):
                nc.tensor.matmul(
                    ps[:],
                    lhsT=WkH[:, cc, :],
                    rhs=xT[:, cc, ts(nk, 512)],
                    start=(cc == 0),
                    stop=(cc == 6),
                )
            nc.vector.tensor_copy(KT[:, ts(nk, 512)], ps[:])
            ps = pp.tile([P, 512], F32, tag="pp")
            for cc in range(7):
                nc.tensor.matmul(
                    ps[:],
                    lhsT=WvH[:, cc, :],
                    rhs=xT[:, cc, ts(nk, 512)],
                    start=(cc == 0),
                    stop=(cc == 6),
                )
            vts = vtt.tile([P, 512], BF16, tag="vts")
            nc.vector.tensor_copy(vts[:], ps[:])
            vps = pp.tile([P, 4, P], BF16, tag="pp")
            for k in range(4):
                nc.tensor.transpose(vps[:, k, :], vts[:, ts(k, P)], identb[:])
            nc.vector.tensor_copy(
                V[:, 4 * nk : 4 * nk + 4, :, 0:32],
                vps[:].rearrange("p k (g e) -> p k g e", g=GL),
            )

            # ---- out-projection for the previous q-chunk ----
            if step >= 1:
                out_proj(step - 1)

            # ---- attention for q-chunk qc = step ----
            qc = step
            qs = qc * 512
            AOn = aop.tile([P, 4, 4, GL, D], BF16, tag="AOn")  # [q, qsub, c, g, d]
            # zero the unused (c>=2, g=3) rows so XBAR input is initialized
            nc.gpsimd.memset(AOn[:, :, 2, 3, :], 0.0)
            nc.gpsimd.memset(AOn[:, :, 3, 3, :], 0.0)
            for g, c0 in PAIRS:
                av = pav.tile([P, 2, 4, 33], F32, tag="av")
                nks = 4 * qc + 4
                for ki in range(nks):
                    ks = ki * P
                    qoff = max(0, ks - qs)
                    sp = pss.tile([P, 2, 512], F32, tag="sp")
                    for j2 in range(2):
                        nc.tensor.matmul(
                            sp[:, j2, qoff:512],
                            lhsT=KT[ts(g, D), ks : ks + P],
                            rhs=QT[ts(g, D), c0 + j2, qs + qoff : qs + 512],
                            start=True,
                            stop=True,
                            tile_position=(g * D, 0),
                        )
                    if ks >= qs:
                        for j2 in range(2):
                            nc.tensor.matmul(
                                sp[:, j2, qoff : qoff + P],
                                lhsT=maskT[:],
                                rhs=identb[:],
                                start=False,
                                stop=True,
                                skip_group_check=True,
                            )
                    pt = ptp.tile([P, 2, 512], BF16, tag="pt")
                    nc.scalar.activation(
                        pt[:, :, qoff:512], sp[:, :, qoff:512], EXP, scale=SCALE
                    )
                    if "dbg_pt" in d and qc == 1 and (g, c0) == (0, 0) and ki == 0:
                        nc.sync.dma_start(
                            d["dbg_pt"][:], pt[:].rearrange("p a b -> p (a b)")
                        )
                    q0 = max(0, ki - 4 * qc)
                    for j2 in range(2):
                        for qsub in range(q0, 4):
                            # start=True only on the block's first matmul: it
                            # marks the whole PSUM zero-region pending-zero, so
                            # each group's first write lands fresh and later
                            # writes accumulate (hw lazy-zero semantics).
                            nc.tensor.matmul(
                                av[:, j2, qsub, :],
                                lhsT=pt[:, j2, ts(qsub, P)],
                                rhs=V[:, ki, g, :],
                                start=(ki == 0 and j2 == 0 and qsub == 0),
                                stop=(ki == nks - 1 and j2 == 1 and qsub == 3),
                                skip_group_check=True,
                            )
                rz = rzp.tile([P, 2, 4], F32, tag="rz")
                nc.vector.reciprocal_approx_fast(rz[:], av[:, :, :, 32])
                for j2 in range(2):
                    nc.vector.tensor_tensor(
                        AOn[:, :, c0 + j2, g, :],
                        av[:, j2, :, 0:32],
                        rz[:, j2, :, None].to_broadcast((P, 4, D)),
                        MUL,
                    )
            # PE transpose AOn -> AOT
            for qsub in range(4):
                aps = pp.tile([P, 4, P], BF16, tag="pp")
                for c in range(4):
                    nc.tensor.transpose(
                        aps[:, c, :],
                        AOn[:, qsub, c, :, :].rearrange("p g e -> p (g e)"),
                        identb[:],
                    )
                nc.vector.tensor_copy(
                    AOT[:, :, qs + qsub * P : qs + (qsub + 1) * P], aps[:]
                )
        out_proj(3)
        if "dbg_qt" in d:
            nc.sync.dma_start(d["dbg_qt"][:], QT[:].rearrange("p c t -> p (c t)"))
            nc.sync.dma_start(d["dbg_kt"][:], KT[:])
            nc.sync.dma_start(d["dbg_v"][:], V[:].rearrange("p a g e -> p (a g e)"))
            nc.sync.dma_start(d["dbg_aot"][:], AOT[:].rearrange("p c t -> p (c t)"))
            nc.sync.dma_start(d["dbg_xt"][:], xT[:].rearrange("p c t -> p (c t)"))


_NC_CACHE = None
DEBUG = False


def _build():
    global _NC_CACHE
    if _NC_CACHE is not None:
        return _NC_CACHE
    nc = bacc.Bacc("TRN2", target_bir_lowering=False, debug=False, num_devices=8)
    d = {
        "x": nc.dram_tensor("x", (T, C), F32R, kind="ExternalInput"),
        "wq": nc.dram_tensor("wq", (C, DH), F32R, kind="ExternalInput"),
        "wk": nc.dram_tensor("wk", (C, DKV), F32R, kind="ExternalInput"),
        "wv": nc.dram_tensor("wv", (C, DKV), F32R, kind="ExternalInput"),
        "wo": nc.dram_tensor("wo", (DH, C), BF16, kind="ExternalInput"),
        "identr": nc.dram_tensor("identr", (P, P), F32R, kind="ExternalInput"),
        "identb": nc.dram_tensor("identb", (P, P), BF16, kind="ExternalInput"),
        "masktb": nc.dram_tensor("masktb", (P, P), BF16, kind="ExternalInput"),
        "out": nc.dram_tensor("out", (T, C), F32, kind="ExternalOutput"),
    }
    if DEBUG:
        d["dbg_qt"] = nc.dram_tensor("dbg_qt", (P, 4 * T), BF16, kind="ExternalOutput")
        d["dbg_kt"] = nc.dram_tensor("dbg_kt", (P, T), BF16, kind="ExternalOutput")
        d["dbg_v"] = nc.dram_tensor("dbg_v", (P, 16 * GL * 33), BF16, kind="ExternalOutput")
        d["dbg_aot"] = nc.dram_tensor("dbg_aot", (P, 4 * T), BF16, kind="ExternalOutput")
        d["dbg_xt"] = nc.dram_tensor("dbg_xt", (P, 7 * T), F32R, kind="ExternalOutput")
        d["dbg_pt"] = nc.dram_tensor("dbg_pt", (P, 2 * 512), BF16, kind="ExternalOutput")
    with tile.TileContext(nc) as tc:
        _trace(tc, {k: v[:] for k, v in d.items()})
    nc.compile()
    _NC_CACHE = nc
    return nc


def _head_cols(hf):
    """Permuted head order: c-major, g-minor (head (g,c) -> global head id)."""
    order = []
    for c in range(4):
        for g in range(4):
            if g == 3 and c >= 2:
                continue
            if g < 3:
                gh = HEADS_HALF[hf][4 * g + c]
            else:
                gh = HEADS_HALF[hf][12 + c]
            order.append(gh)
    return np.concatenate([np.arange(32 * h, 32 * h + 32) for h in order])


def _in_maps(x, Wq, Wk, Wv, Wo):
    import ml_dtypes

    bf16 = np.dtype(ml_dtypes.bfloat16)
    identr = np.eye(P, dtype=np.float32)
    identb_bits = np.eye(P, dtype=np.float32).astype(bf16)
    masktb = np.where(
        np.arange(P)[:, None] < np.arange(P)[None, :], np.float32(NEG), np.float32(0)
    )
    masktb_bits = masktb.astype(bf16)
    maps = []
    for cidx in range(8):
        b, hf = cidx // 2, cidx % 2
        hcols = _head_cols(hf)
        kcols = np.concatenate([np.arange(32 * g, 32 * g + 32) for g in KV_HALF[hf]])
        maps.append(
            {
                "x": np.ascontiguousarray(x[b]),
                "wq": np.ascontiguousarray(Wq[:, hcols]),
                "wk": np.ascontiguousarray(Wk[:, kcols]),
                "wv": np.ascontiguousarray(Wv[:, kcols]),
                "wo": np.ascontiguousarray(Wo[hcols, :]).astype(bf16),
                "identr": identr,
                "identb": identb_bits,
                "masktb": masktb_bits,
            }
        )
    return maps


def run(x, Wq, Wk, Wv, Wo, trace=False):
    nc = _build()
    res = run_bass_kernel_spmd(
        nc, _in_maps(x, Wq, Wk, Wv, Wo), core_ids=list(range(8)), trace=trace
    )
    outs = [r["out"] for r in res.results]
    final = np.empty((4, T, C), np.float32)
    for b in range(4):
        final[b] = outs[2 * b] + outs[2 * b + 1]
    return final, res


def kernel(x, Wq, Wk, Wv, Wo):
    x = np.asarray(x, dtype=np.float32)
    out, _ = run(
        x,
        np.asarray(Wq, np.float32),
        np.asarray(Wk, np.float32),
        np.asarray(Wv, np.float32),
        np.asarray(Wo, np.float32),
    )
    return out
